# revision 46
# baseline (speedup 1.0000x reference)
"""MixLinear (int8-quantized GEMM + fp16 outlier GEMM) Trainium2 kernel.

Row-parallel across 8 NeuronCores: core c computes output rows
[c*1024, (c+1)*1024) of the flattened [8192, 11008] output. x rows are
sharded; weight is replicated (streamed from DRAM once per core).

Math performed on device per core (M=1024 local rows, K=4096, N=11008):
  xz      = x * mask                      (zero outlier columns)
  amax    = max(|xz|) per row
  xs      = max(amax/127, 1e-8); inv = 1/xs
  q       = round(xz * inv)               (fp16 magic-number rounding)
  qxs     = q * xs                        (fp16, exact int scaled back)
  psum    = ones x bias                   (rank-1 bias seed)
          + qxs @ (W*scale_col)^T         (fp16 matmul, fp32 accumulate)
          + x[:, ind] @ weight_cache^T    (outlier matmul, same psum)
  out     = fp16(psum)

Host-side prep (cheap, index/layout only): transpose+scale weight to
(W*sc)^T fp16, transpose weight_cache, build the zero-mask from ind.
Outlier activation columns are gathered on-chip with per-column engine
copies (split across ACT/DVE) and PE-transposed into [F, M] layout.
"""

import sys

sys.path.insert(0, "/opt/trn_rl_repo")

from contextlib import ExitStack

import numpy as np

import concourse.bass as bass
import concourse.tile as tile
from concourse import bacc, mybir
from concourse.bass_utils import run_bass_kernel_spmd
from concourse.masks import make_identity

B, S, K, N, F = 4, 2048, 4096, 11008, 128
NCORES = 8
M = B * S
M_LOC = M // NCORES
P = 128

FP16 = mybir.dt.float16
FP32 = mybir.dt.float32

MAGIC = 1536.0  # fp16 ulp == 1 in [1024, 2048): adding forces round-to-int
WB = 8  # weight k-chunks batched per DMA

_EXEC_TIME_NS = None
_BUILD_CACHE = {}


def _build(ind_host, m_loc=M_LOC, k=K, n=N, f=F):
    """Build + compile the per-core Tile program. ind_host: python ints."""
    kc = k // P  # number of 128-wide K chunks
    wb = min(WB, kc)  # weight chunks per DMA batch
    mt = m_loc // P  # number of 128-row M tiles
    n_sizes = []
    left = n
    while left > 0:
        n_sizes.append(min(512, left))
        left -= 512

    nc = bacc.Bacc(
        "TRN2",
        target_bir_lowering=False,
        debug=False,
        enable_asserts=False,
        num_devices=NCORES,
    )

    xs_d = nc.dram_tensor("xs", [m_loc, k], FP16, kind="ExternalInput").ap()
    wT_d = nc.dram_tensor("wT", [k, n], FP16, kind="ExternalInput").ap()
    bias_d = nc.dram_tensor("biasf", [1, n], FP16, kind="ExternalInput").ap()
    mask_d = nc.dram_tensor("maskf", [1, k], FP16, kind="ExternalInput").ap()
    out_d = nc.dram_tensor("out", [m_loc, n], FP16, kind="ExternalOutput").ap()

    # weight viewed as [p, chunk-batch, n] for batched chunk loads
    wT_v = wT_d.rearrange("(cb p) n -> p cb n", p=P)

    with tile.TileContext(nc) as tc, ExitStack() as ctx:
        const = ctx.enter_context(tc.tile_pool(name="const", bufs=1))
        res = ctx.enter_context(tc.tile_pool(name="res", bufs=1))
        pha = ctx.enter_context(tc.tile_pool(name="pha", bufs=2))
        wpool = ctx.enter_context(tc.tile_pool(name="wp", bufs=2 * (kc // wb)))
        bpool = ctx.enter_context(tc.tile_pool(name="bp", bufs=2))
        opool = ctx.enter_context(tc.tile_pool(name="op", bufs=4))
        ps_t = ctx.enter_context(tc.tile_pool(name="ps_t", bufs=2, space="PSUM"))
        ps_mm = ctx.enter_context(tc.tile_pool(name="ps_mm", bufs=4, space="PSUM"))
        ps_b = ctx.enter_context(tc.tile_pool(name="ps_b", bufs=2, space="PSUM"))

        identity = const.tile([P, P], FP16)
        make_identity(nc, identity[:])
        ones_t = const.tile([1, P], FP16)
        nc.vector.memset(ones_t[:], 1.0)
        mask_sb = const.tile([P, k], FP16)
        nc.gpsimd.dma_start(out=mask_sb[:], in_=mask_d.to_broadcast([P, k]))

        # Resident transposed tensors
        qxsT = res.tile([P, kc, m_loc], FP16)  # [k-chunk][k_in, m]
        xs_col = res.tile([P, mt], FP32)  # per-row x_scale, col per m-tile

        # ---- Phase A: quantization + outlier gather (per 128-row m-tile) ----
        for t in range(mt):
            msl = bass.ds(t * P, P)
            kh = k // 2
            # two half-tiles loaded in parallel on both HW queues
            xh0 = pha.tile([P, kh], FP16, tag="xt0", bufs=3)
            xh1 = pha.tile([P, kh], FP16, tag="xt1", bufs=3)
            xh = [xh0, xh1]
            nc.scalar.dma_start(out=xh0[:], in_=xs_d[msl, :kh])
            nc.sync.dma_start(out=xh1[:], in_=xs_d[msl, kh:])

            # amax = absmax(x*mask) per row; masked product split DVE/GPSIMD
            # per half, reduces combined at the end.
            # Quantization below reads raw x: outlier columns of q carry
            # (quantized) activations, and the host writes weight_cache rows
            # into wT's outlier rows, so the main GEMM also computes the
            # outlier contribution -- no separate gather/outlier matmul.
            red = []
            for h in range(2):
                xz = pha.tile([P, kh], FP16, tag=f"xz{h}", bufs=1)
                # tile 0 is on the kernel's critical path: keep its chain off
                # the slow GPSIMD tensor_tensor
                eng = nc.vector if (h == 0 or t == 0) else nc.gpsimd
                eng.tensor_mul(xz[:], xh[h][:], mask_sb[:, h * kh : (h + 1) * kh])
                r = pha.tile([P, 1], FP32, tag=f"r{h}")
                nc.vector.tensor_reduce(
                    out=r[:], in_=xz[:], axis=mybir.AxisListType.X,
                    op=mybir.AluOpType.max, apply_absolute_value=True,
                )
                red.append(r)
            amax = pha.tile([P, 1], FP32, tag="amax")
            nc.vector.tensor_max(amax[:], red[0][:], red[1][:])
            nc.vector.tensor_scalar(
                out=xs_col[:, t : t + 1],
                in0=amax[:],
                scalar1=1.0 / 127.0,
                scalar2=1e-8,
                op0=mybir.AluOpType.mult,
                op1=mybir.AluOpType.max,
            )
            inv = pha.tile([P, 1], FP32, tag="inv")
            nc.vector.reciprocal(inv[:], xs_col[:, t : t + 1])
            negmxs = pha.tile([P, 1], FP32, tag="negmxs")
            nc.vector.tensor_scalar(
                out=negmxs[:],
                in0=xs_col[:, t : t + 1],
                scalar1=-MAGIC,
                scalar2=None,
                op0=mybir.AluOpType.mult,
            )
            for h in range(2):
                # q16 = round(x*inv) + MAGIC  (round happens at fp16 writeback)
                q16 = pha.tile([P, kh], FP16, tag=f"q16{h}", bufs=1)
                nc.vector.tensor_scalar(
                    out=q16[:],
                    in0=xh[h][:],
                    scalar1=inv[:],
                    scalar2=MAGIC,
                    op0=mybir.AluOpType.mult,
                    op1=mybir.AluOpType.add,
                )
                # qxs = q16*xs - MAGIC*xs  (on ACT: Identity(scale*x + bias);
                # DVE dual-op for tile 0 -- ACT is ~2x slower per pass)
                qq = pha.tile([P, kh], FP16, tag=f"qq{h}", bufs=1)
                if t == 0:
                    nc.vector.tensor_scalar(
                        out=qq[:],
                        in0=q16[:],
                        scalar1=MAGIC,
                        scalar2=xs_col[:, t : t + 1],
                        op0=mybir.AluOpType.subtract,
                        op1=mybir.AluOpType.mult,
                    )
                else:
                    nc.scalar.activation(
                        out=qq[:],
                        in_=q16[:],
                        func=mybir.ActivationFunctionType.Identity,
                        bias=negmxs[:],
                        scale=xs_col[:, t : t + 1],
                    )
                # Transpose [128, 128] chunks via PE, 8 per PSUM bank, then
                # one batched evacuation copy per bank into resident qxsT
                kch = kh // P
                for cb in range((kch + 7) // 8):
                    cn = min(8, kch - cb * 8)
                    pt = ps_t.tile([P, 8 * P], FP16, tag="pt")
                    for ci in range(cn):
                        c = cb * 8 + ci
                        nc.tensor.transpose(
                            pt[:, bass.ds(ci * P, P)],
                            qq[:, bass.ds(c * P, P)],
                            identity[:],
                        )
                    nc.scalar.copy(
                        qxsT[:, bass.ds(h * kch + cb * 8, cn), msl],
                        pt[:, : cn * P],
                    )

        # ---- Main loop: N tiles x M tiles ----
        n0 = 0
        for nt, nw in enumerate(n_sizes):
            nsl = bass.ds(n0, nw)
            wts = []
            for cb in range(kc // wb):
                wt = wpool.tile([P, wb, 512], FP16, tag="w")
                deng = nc.sync if cb % 2 == 0 else nc.scalar
                deng.dma_start(
                    out=wt[:, :, :nw], in_=wT_v[:, bass.ds(cb * wb, wb), nsl]
                )
                wts.append(wt)
            bias_sb = bpool.tile([1, 512], FP16, tag="bias")
            nc.sync.dma_start(out=bias_sb[:, :nw], in_=bias_d[:, nsl])
            # broadcast bias to all partitions once per N tile (rank-1 PE)
            psb = ps_b.tile([P, 512], FP32, tag="psb")
            nc.tensor.matmul(psb[:, :nw], ones_t[:], bias_sb[:, :nw])
            bias_bc = bpool.tile([P, 512], FP32, tag="bias_bc")
            nc.scalar.copy(bias_bc[:, :nw], psb[:, :nw])
            for t in range(mt):
                msl = bass.ds(t * P, P)
                ps = ps_mm.tile([P, 512], FP32, tag="ps")
                for c in range(kc):
                    nc.tensor.matmul(
                        ps[:, :nw],
                        qxsT[:, c, msl],
                        wts[c // wb][:, c % wb, :nw],
                        start=(c == 0),
                        stop=(c == kc - 1),
                    )
                ot = opool.tile([P, 512], FP16, tag="ot")
                nc.vector.tensor_add(ot[:, :nw], ps[:, :nw], bias_bc[:, :nw])
                nc.sync.dma_start(out=out_d[msl, nsl], in_=ot[:, :nw])
            n0 += nw

    nc.compile()
    return nc


def kernel(x, weight, scale_col, weight_cache, ind, bias):
    global _EXEC_TIME_NS
    x = np.asarray(x)
    weight = np.asarray(weight)
    scale_col = np.asarray(scale_col)
    weight_cache = np.asarray(weight_cache)
    ind = np.asarray(ind)
    bias = np.asarray(bias)

    b, s, k = x.shape
    n = weight.shape[0]
    xf = np.ascontiguousarray(x.reshape(-1, k))

    ind_host = tuple(int(v) for v in ind)
    mask = np.ones((1, k), np.float16)
    mask[0, list(ind_host)] = np.float16(0)

    # (W * scale_col)^T in fp16, [K, N]
    w_sc = (weight.astype(np.float32) * scale_col.reshape(n, 1).astype(np.float32)).astype(
        np.float16
    )
    wT = np.ascontiguousarray(w_sc.T)
    del w_sc
    # Outlier rows of wT carry weight_cache instead of the scaled int8
    # weights: on-device q keeps (quantized) activations at outlier columns,
    # so the main GEMM computes the outlier contribution in the same pass.
    wT[list(ind_host), :] = weight_cache.astype(np.float16).T
    biasf = np.ascontiguousarray(bias.astype(np.float16).reshape(1, n))

    key = (ind_host, x.shape)
    if key not in _BUILD_CACHE:
        _BUILD_CACHE.clear()
        _BUILD_CACHE[key] = _build(ind_host)
    nc = _BUILD_CACHE[key]

    m_loc = xf.shape[0] // NCORES
    in_maps = [
        {
            "xs": np.ascontiguousarray(xf[c * m_loc : (c + 1) * m_loc]),
            "wT": wT,
            "biasf": biasf,
            "maskf": mask,
        }
        for c in range(NCORES)
    ]

    res = run_bass_kernel_spmd(nc, in_maps, list(range(NCORES)))
    _EXEC_TIME_NS = res.exec_time_ns
    out = np.concatenate([res.results[c]["out"] for c in range(NCORES)], axis=0)
    return out.reshape(b, s, n)


# revision 49
# speedup vs baseline: 1.2046x; 1.2046x over previous
"""MixLinear (int8-quantized GEMM + fp16 outlier GEMM) Trainium2 kernel.

Row-parallel across 8 NeuronCores: core c computes output rows
[c*1024, (c+1)*1024) of the flattened [8192, 11008] output. x rows are
sharded; weight is replicated (streamed from DRAM once per core).

Math performed on device per core (M=1024 local rows, K=4096, N=11008):
  xz      = x * mask                      (zero outlier columns)
  amax    = max(|xz|) per row
  xs      = max(amax/127, 1e-8); inv = 1/xs
  q       = round(xz * inv)               (fp16 magic-number rounding)
  qxs     = q * xs                        (fp16, exact int scaled back)
  psum    = ones x bias                   (rank-1 bias seed)
          + qxs @ (W*scale_col)^T         (fp16 matmul, fp32 accumulate)
          + x[:, ind] @ weight_cache^T    (outlier matmul, same psum)
  out     = fp16(psum)

Host-side prep (cheap, index/layout only): transpose+scale weight to
(W*sc)^T fp16, transpose weight_cache, build the zero-mask from ind.
Outlier activation columns are gathered on-chip with per-column engine
copies (split across ACT/DVE) and PE-transposed into [F, M] layout.
"""

import sys

sys.path.insert(0, "/opt/trn_rl_repo")

from contextlib import ExitStack

import numpy as np

import concourse.bass as bass
import concourse.tile as tile
from concourse import bacc, mybir
from concourse.bass_utils import run_bass_kernel_spmd
from concourse.masks import make_identity

B, S, K, N, F = 4, 2048, 4096, 11008, 128
NCORES = 8
M = B * S
M_LOC = M // NCORES
P = 128

FP16 = mybir.dt.float16
FP32 = mybir.dt.float32

MAGIC = 1536.0  # fp16 ulp == 1 in [1024, 2048): adding forces round-to-int
WB = 8  # weight k-chunks batched per DMA

_EXEC_TIME_NS = None
_BUILD_CACHE = {}


def _build(ind_host, m_loc=M_LOC, k=K, n=N, f=F):
    """Build + compile the per-core Tile program. ind_host: python ints."""
    kc = k // P  # number of 128-wide K chunks
    wb = min(WB, kc)  # weight chunks per DMA batch
    mt = m_loc // P  # number of 128-row M tiles
    n_sizes = []
    left = n
    while left > 0:
        n_sizes.append(min(512, left))
        left -= 512

    nc = bacc.Bacc(
        "TRN2",
        target_bir_lowering=False,
        debug=False,
        enable_asserts=False,
        num_devices=NCORES,
    )

    xs_d = nc.dram_tensor("xs", [m_loc, k], FP16, kind="ExternalInput").ap()
    wT_d = nc.dram_tensor("wT", [k, n], FP16, kind="ExternalInput").ap()
    bias_d = nc.dram_tensor("biasf", [1, n], FP16, kind="ExternalInput").ap()
    mask_d = nc.dram_tensor("maskf", [1, k], FP16, kind="ExternalInput").ap()
    out_d = nc.dram_tensor("out", [m_loc, n], FP16, kind="ExternalOutput").ap()

    # weight viewed as [p, chunk-batch, n] for batched chunk loads
    wT_v = wT_d.rearrange("(cb p) n -> p cb n", p=P)

    with tile.TileContext(nc) as tc, ExitStack() as ctx:
        const = ctx.enter_context(tc.tile_pool(name="const", bufs=1))
        res = ctx.enter_context(tc.tile_pool(name="res", bufs=1))
        pha = ctx.enter_context(tc.tile_pool(name="pha", bufs=2))
        wpool = ctx.enter_context(tc.tile_pool(name="wp", bufs=2 * (kc // wb) - 2))
        bpool = ctx.enter_context(tc.tile_pool(name="bp", bufs=2))
        opool = ctx.enter_context(tc.tile_pool(name="op", bufs=4))
        ps_t = ctx.enter_context(tc.tile_pool(name="ps_t", bufs=2, space="PSUM"))
        ps_mm = ctx.enter_context(tc.tile_pool(name="ps_mm", bufs=4, space="PSUM"))
        ps_b = ctx.enter_context(tc.tile_pool(name="ps_b", bufs=2, space="PSUM"))

        identity = const.tile([P, P], FP16)
        make_identity(nc, identity[:])
        ones_t = const.tile([1, P], FP16)
        nc.vector.memset(ones_t[:], 1.0)
        mask_sb = const.tile([P, k], FP16)
        nc.gpsimd.dma_start(out=mask_sb[:], in_=mask_d.to_broadcast([P, k]))

        # Resident transposed tensors
        qxsT = res.tile([P, kc, m_loc], FP16)  # [k-chunk][k_in, m]
        xs_col = res.tile([P, mt], FP32)  # per-row x_scale, col per m-tile

        # ---- Phase A: quantization + outlier gather (per 128-row m-tile) ----
        for t in range(mt):
            msl = bass.ds(t * P, P)
            kh = k // 2
            # two half-tiles loaded in parallel on both HW queues
            xh0 = pha.tile([P, kh], FP16, tag="xt0", bufs=4)
            xh1 = pha.tile([P, kh], FP16, tag="xt1", bufs=4)
            xh = [xh0, xh1]
            nc.scalar.dma_start(out=xh0[:], in_=xs_d[msl, :kh])
            nc.sync.dma_start(out=xh1[:], in_=xs_d[msl, kh:])

            # amax = absmax(x*mask) per row; masked product split DVE/GPSIMD
            # per half, reduces combined at the end.
            # Quantization below reads raw x: outlier columns of q carry
            # (quantized) activations, and the host writes weight_cache rows
            # into wT's outlier rows, so the main GEMM also computes the
            # outlier contribution -- no separate gather/outlier matmul.
            red = []
            for h in range(2):
                xz = pha.tile([P, kh], FP16, tag=f"xz{h}", bufs=1)
                # tile 0 is on the kernel's critical path: keep its chain off
                # the slow GPSIMD tensor_tensor
                eng = nc.vector if (h == 0 or t == 0) else nc.gpsimd
                eng.tensor_mul(xz[:], xh[h][:], mask_sb[:, h * kh : (h + 1) * kh])
                r = pha.tile([P, 1], FP32, tag=f"r{h}")
                nc.vector.tensor_reduce(
                    out=r[:], in_=xz[:], axis=mybir.AxisListType.X,
                    op=mybir.AluOpType.max, apply_absolute_value=True,
                )
                red.append(r)
            amax = pha.tile([P, 1], FP32, tag="amax")
            nc.vector.tensor_max(amax[:], red[0][:], red[1][:])
            nc.vector.tensor_scalar(
                out=xs_col[:, t : t + 1],
                in0=amax[:],
                scalar1=1.0 / 127.0,
                scalar2=1e-8,
                op0=mybir.AluOpType.mult,
                op1=mybir.AluOpType.max,
            )
            inv = pha.tile([P, 1], FP32, tag="inv")
            nc.vector.reciprocal(inv[:], xs_col[:, t : t + 1])
            negmxs = pha.tile([P, 1], FP32, tag="negmxs")
            nc.vector.tensor_scalar(
                out=negmxs[:],
                in0=xs_col[:, t : t + 1],
                scalar1=-MAGIC,
                scalar2=None,
                op0=mybir.AluOpType.mult,
            )
            for h in range(2):
                # q16 = round(x*inv) + MAGIC  (round happens at fp16 writeback)
                q16 = pha.tile([P, kh], FP16, tag=f"q16{h}", bufs=1)
                nc.vector.tensor_scalar(
                    out=q16[:],
                    in0=xh[h][:],
                    scalar1=inv[:],
                    scalar2=MAGIC,
                    op0=mybir.AluOpType.mult,
                    op1=mybir.AluOpType.add,
                )
                # qxs = q16*xs - MAGIC*xs  (on ACT: Identity(scale*x + bias);
                # DVE dual-op for tile 0 -- ACT is ~2x slower per pass)
                qq = pha.tile([P, kh], FP16, tag=f"qq{h}", bufs=1)
                if t == 0:
                    nc.vector.tensor_scalar(
                        out=qq[:],
                        in0=q16[:],
                        scalar1=MAGIC,
                        scalar2=xs_col[:, t : t + 1],
                        op0=mybir.AluOpType.subtract,
                        op1=mybir.AluOpType.mult,
                    )
                else:
                    nc.scalar.activation(
                        out=qq[:],
                        in_=q16[:],
                        func=mybir.ActivationFunctionType.Identity,
                        bias=negmxs[:],
                        scale=xs_col[:, t : t + 1],
                    )
                # Transpose [128, 128] chunks via PE, 8 per PSUM bank, then
                # one batched evacuation copy per bank into resident qxsT
                kch = kh // P
                for cb in range((kch + 7) // 8):
                    cn = min(8, kch - cb * 8)
                    pt = ps_t.tile([P, 8 * P], FP16, tag="pt")
                    for ci in range(cn):
                        c = cb * 8 + ci
                        nc.tensor.transpose(
                            pt[:, bass.ds(ci * P, P)],
                            qq[:, bass.ds(c * P, P)],
                            identity[:],
                        )
                    nc.scalar.copy(
                        qxsT[:, bass.ds(h * kch + cb * 8, cn), msl],
                        pt[:, : cn * P],
                    )

        # ---- Main loop: N tiles x M tiles ----
        n0 = 0
        for nt, nw in enumerate(n_sizes):
            nsl = bass.ds(n0, nw)
            wts = []
            for cb in range(kc // wb):
                wt = wpool.tile([P, wb, 512], FP16, tag="w")
                deng = nc.sync if cb % 2 == 0 else nc.scalar
                deng.dma_start(
                    out=wt[:, :, :nw], in_=wT_v[:, bass.ds(cb * wb, wb), nsl]
                )
                wts.append(wt)
            bias_sb = bpool.tile([1, 512], FP16, tag="bias")
            nc.sync.dma_start(out=bias_sb[:, :nw], in_=bias_d[:, nsl])
            # broadcast bias to all partitions once per N tile (rank-1 PE)
            psb = ps_b.tile([P, 512], FP32, tag="psb")
            nc.tensor.matmul(psb[:, :nw], ones_t[:], bias_sb[:, :nw])
            bias_bc = bpool.tile([P, 512], FP32, tag="bias_bc")
            nc.scalar.copy(bias_bc[:, :nw], psb[:, :nw])
            for t in range(mt):
                msl = bass.ds(t * P, P)
                ps = ps_mm.tile([P, 512], FP32, tag="ps")
                for c in range(kc):
                    nc.tensor.matmul(
                        ps[:, :nw],
                        qxsT[:, c, msl],
                        wts[c // wb][:, c % wb, :nw],
                        start=(c == 0),
                        stop=(c == kc - 1),
                    )
                ot = opool.tile([P, 512], FP16, tag="ot")
                nc.vector.tensor_add(ot[:, :nw], ps[:, :nw], bias_bc[:, :nw])
                nc.sync.dma_start(out=out_d[msl, nsl], in_=ot[:, :nw])
            n0 += nw

    nc.compile()
    return nc


def kernel(x, weight, scale_col, weight_cache, ind, bias):
    global _EXEC_TIME_NS
    x = np.asarray(x)
    weight = np.asarray(weight)
    scale_col = np.asarray(scale_col)
    weight_cache = np.asarray(weight_cache)
    ind = np.asarray(ind)
    bias = np.asarray(bias)

    b, s, k = x.shape
    n = weight.shape[0]
    xf = np.ascontiguousarray(x.reshape(-1, k))

    ind_host = tuple(int(v) for v in ind)
    mask = np.ones((1, k), np.float16)
    mask[0, list(ind_host)] = np.float16(0)

    # (W * scale_col)^T in fp16, [K, N]
    w_sc = (weight.astype(np.float32) * scale_col.reshape(n, 1).astype(np.float32)).astype(
        np.float16
    )
    wT = np.ascontiguousarray(w_sc.T)
    del w_sc
    # Outlier rows of wT carry weight_cache instead of the scaled int8
    # weights: on-device q keeps (quantized) activations at outlier columns,
    # so the main GEMM computes the outlier contribution in the same pass.
    wT[list(ind_host), :] = weight_cache.astype(np.float16).T
    biasf = np.ascontiguousarray(bias.astype(np.float16).reshape(1, n))

    key = (ind_host, x.shape)
    if key not in _BUILD_CACHE:
        _BUILD_CACHE.clear()
        _BUILD_CACHE[key] = _build(ind_host)
    nc = _BUILD_CACHE[key]

    m_loc = xf.shape[0] // NCORES
    in_maps = [
        {
            "xs": np.ascontiguousarray(xf[c * m_loc : (c + 1) * m_loc]),
            "wT": wT,
            "biasf": biasf,
            "maskf": mask,
        }
        for c in range(NCORES)
    ]

    res = run_bass_kernel_spmd(nc, in_maps, list(range(NCORES)))
    _EXEC_TIME_NS = res.exec_time_ns
    out = np.concatenate([res.results[c]["out"] for c in range(NCORES)], axis=0)
    return out.reshape(b, s, n)
